# revision 23
# baseline (speedup 1.0000x reference)
"""MetaSR (nn_MetaSR_74517682585959) Trainium2 Bass kernel.

Strategy (8 NeuronCores, query-parallel):
 - Replicate encoder+MLP params + feature volume on every core; shard the
   200k queries 8 ways (25000 + pad -> 25088 = 49*512 per core).
 - The metric (warm dispatch wall time over the axon tunnel) is dominated
   by per-call host work, not device compute (~2 ms), so the design
   minimizes (a) input upload bytes, (b) BIR instruction count (walrus
   recompiles on every dispatch).
   a. Inputs: one f16 const blob (padded 36^3 volume + all weights,
      880 KB) uploaded as 1/8th shards and AllGathered on-device, plus a
      per-core f32 blob ([q,4] packed coords + biases, 407 KB). Output
      f16. ~0.57 MB/core/dispatch vs 10.9 MB for the im2col-upload
      baseline.
   b. For_i hardware loops over the 7 query macro-tiles and the 30
      interior z-slices: 1.2k BIR instructions vs 5.8k unrolled.
 - Per-core flow:
   1. Build the 5x5x5 im2col x2h[126, 32, 1024] f16 in DRAM from the
      padded volume via 125 strided window DMAs (row 125 = ones row for
      the conv bias). Voxel order within a z-slice is natural col=y*32+x.
   2. Build the unfolded-feature table T[32768 voxels, 512 ch] f16 in
      DRAM via one K=126 matmul per 128-voxel block (conv3x3 o unfold3x3
      collapsed, bias via the ones row). Unfold zero-padding: x-edges via
      a mask-multiply PSUM evacuation (p%32==0 / p%32==31 rows), y-edges
      via partition-range memsets on blk 0/7, z-edges via channel-range
      memsets. Mask + identity are built on-chip (no upload).
   3. Per 3584-query macro-tile: voxel indices / rel coords on DVE
      (q_coord is analytic - no second gather), gather q_feat^T via
      transpose-mode dma_gather (f16, channel-major out [128, 4, 512]).
   4. MLP 4->256->256->256->256->512(=permuted padded 432) in f16 on PE
      (f32 psum); ReLU+bias fused into PSUM evacuation (ACT/DVE).
   5. out[q] = sum_ch qf*pred: f16xf16->f32 products (DVE), then
      partition-reduce with a ones-vector matmul on PE, software-
      pipelined one sub-tile behind the MLP within each macro body.
"""

import numpy as np

QTOT = 200000
NCORES = 8
QPC = QTOT // NCORES          # 25000
QPAD = 25088                  # 49 * 512
MACRO = 3584                  # 28 * 128 queries per macro tile
NMACRO = QPAD // MACRO        # 7
NSUB = MACRO // 512           # 7
COLS = MACRO // 128           # 28

# f16 blob layout (element offsets)
OFF_P36 = 0
N_P36 = 36 * 36 * 36          # 46656
OFF_W2H = OFF_P36 + N_P36
N_W2H = 126 * 512
OFF_W1 = OFF_W2H + N_W2H
OFF_W2 = OFF_W1 + 4 * 256
OFF_W3 = OFF_W2 + 256 * 256
OFF_W4 = OFF_W3 + 256 * 256
OFF_W5 = OFF_W4 + 256 * 256
N16 = OFF_W5 + 256 * 512      # 439872

# f32 bias blob layout (element offsets)
OFF_B1 = 0
OFF_B2 = OFF_B1 + 256
OFF_B3 = OFF_B2 + 256
OFF_B4 = OFF_B3 + 256
OFF_B5 = OFF_B4 + 256
NBIA = OFF_B5 + 512           # 1536

N16P = N16 // 8               # 54984: per-core AllGather shard of blob16

_COMPILED = {}


def _build_host_consts(inp, W_enc, b_enc, W1, W2, W3, W4, W5, b5):
    """Padded volume, fused conv+unfold weights, f16 MLP weights, w5 perm."""
    v = np.asarray(inp, np.float32)[0, 0]                     # [32,32,32]
    p36 = np.pad(v, 2).astype(np.float16)                     # [36,36,36]

    We = np.asarray(W_enc, np.float32)                        # [16,1,3,3,3]
    w2h = np.zeros((5, 5, 5, 27, 16), np.float32)
    for dz in range(3):
        for dy in range(3):
            for dx in range(3):
                j = dz * 9 + dy * 3 + dx
                for az in range(3):
                    for ay in range(3):
                        for ax in range(3):
                            w2h[dz + az, dy + ay, dx + ax, j, :] = We[:, 0, az, ay, ax]
    w2h_full = np.zeros((126, 512), np.float32)
    w2h_full[:125, :432] = w2h.reshape(125, 432)
    w2h_full[125, :432] = np.tile(np.asarray(b_enc, np.float32), 27)  # j-major bias
    w2h_full = w2h_full.astype(np.float16)

    perm = np.array([c * 27 + j for j in range(27) for c in range(16)], np.int64)
    w5p = np.zeros((256, 512), np.float32)
    w5p[:, :432] = np.asarray(W5, np.float32)[:, perm]
    b5p = np.zeros((512,), np.float32)
    b5p[:432] = np.asarray(b5, np.float32)[perm]

    blob16 = np.concatenate([
        p36.ravel(),
        w2h_full.ravel(),
        np.asarray(W1, np.float32).astype(np.float16).ravel(),
        np.asarray(W2, np.float32).astype(np.float16).ravel(),
        np.asarray(W3, np.float32).astype(np.float16).ravel(),
        np.asarray(W4, np.float32).astype(np.float16).ravel(),
        w5p.astype(np.float16).ravel(),
    ])
    assert blob16.size == N16
    return blob16, b5p


def _patch_tile_drain():
    """Walrus in this toolchain rejects >2 sem waits on the Tile tail drain;
    split the waits across multiple drain instructions."""
    import concourse.mybir as mybir
    from concourse import tile
    from concourse.vector_clock import ScopedClock

    if getattr(tile.TileContext, "_drain_split_patch", False):
        return

    def _drain_and_barrier(self, tick_clock, wait_clock):
        nc = self.nc
        drain_inst = nc.sync.drain()
        wait_clock.add_sem_waits(
            drain_inst.ins, ScopedClock({None: tick_clock.global_clock})
        )
        si = drain_inst.ins.sync_info
        waits = list(si.on_wait) if si is not None else []
        if len(waits) > 1:
            drain_inst.ins.sync_info = mybir.SyncInfo(
                on_wait=waits[:1], on_update=list(si.on_update)
            )
            for w in waits[1:]:
                d2 = nc.sync.drain()
                d2.ins.sync_info = mybir.SyncInfo(on_wait=[w], on_update=[])

    tile.TileContext._drain_and_barrier = _drain_and_barrier
    tile.TileContext._drain_split_patch = True


def build_nc(qpad=QPAD, nmacro=NMACRO, phase="full"):
    import concourse.bass as bass
    import concourse.bacc as bacc
    import concourse.mybir as mybir
    from concourse import tile
    from concourse.bass import ds

    _patch_tile_drain()

    f32 = mybir.dt.float32
    f16 = mybir.dt.float16
    i32 = mybir.dt.int32
    i16 = mybir.dt.int16
    AF = mybir.ActivationFunctionType
    OP = mybir.AluOpType

    macro = MACRO
    nsub = NSUB
    cols = COLS
    assert qpad == nmacro * macro

    nc = bacc.Bacc(None, target_bir_lowering=False, num_devices=NCORES)
    b16p_d = nc.dram_tensor("b16p", [N16P], f16, kind="ExternalInput")
    # per-core query data, host-prepped in device layouts:
    #   idx: wrapped gather indices (the exact idxr SBUF layout per macro)
    #   xin: feature-major MLP inputs [rel*32 (3), cell_x*16] per macro
    idx_d = nc.dram_tensor("idx", [nmacro, 16, cols * 8], i16, kind="ExternalInput")
    xin_d = nc.dram_tensor("xin", [nmacro, 4, macro], f16, kind="ExternalInput")
    bia_d = nc.dram_tensor("bia", [NBIA], f32, kind="ExternalInput")
    out_d = nc.dram_tensor("out", [nmacro, 1, nsub, 512], f16, kind="ExternalOutput")

    bv = {
        "b1": bia_d[OFF_B1:OFF_B2].rearrange("(p o) -> p o", o=1),
        "b2": bia_d[OFF_B2:OFF_B3].rearrange("(p o) -> p o", o=1),
        "b3": bia_d[OFF_B3:OFF_B4].rearrange("(p o) -> p o", o=1),
        "b4": bia_d[OFF_B4:OFF_B5].rearrange("(p o) -> p o", o=1),
        "b5": bia_d[OFF_B5:NBIA].rearrange("(p o) -> p o", o=1),
    }

    with tile.TileContext(nc) as tc:
        with (
            tc.tile_pool(name="dram", bufs=1, space="DRAM") as dpool,
            tc.tile_pool(name="const", bufs=1) as cpool,
        ):
            table = dpool.tile([32, 128, 8, 512], f16)
            tabflat = table[:, :, :, :].rearrange("z p a f -> (z p a) f")
            x2h = dpool.tile([126, 32, 1024], f16)

            # ---- AllGather the replicated const blob (each core uploads
            # 1/8th; collectives can't read I/O tensors, so bounce first) ----
            b16b = dpool.tile([N16P], f16)
            nc.sync.dma_start(b16b[:], b16p_d[:])
            b16g = dpool.tile([N16], f16)
            nc.gpsimd.collective_compute(
                "AllGather", mybir.AluOpType.bypass,
                replica_groups=[list(range(NCORES))],
                ins=[b16b[:]], outs=[b16g[:]],
            )
            p36v = b16g[OFF_P36:OFF_P36 + N_P36].rearrange("(a b c) -> a b c", b=36, c=36)
            w2hv = b16g[OFF_W2H:OFF_W2H + N_W2H].rearrange("(p f) -> p f", f=512)
            w1v = b16g[OFF_W1:OFF_W2].rearrange("(p f) -> p f", f=256)
            wkv = {
                "w2": b16g[OFF_W2:OFF_W3].rearrange("(p f) -> p f", f=256),
                "w3": b16g[OFF_W3:OFF_W4].rearrange("(p f) -> p f", f=256),
                "w4": b16g[OFF_W4:OFF_W5].rearrange("(p f) -> p f", f=256),
                "w5": b16g[OFF_W5:N16].rearrange("(p f) -> p f", f=512),
            }

            # ---- persistent constants in SBUF ----
            w2h = cpool.tile([126, 512], f16)
            nc.sync.dma_start(w2h[:, :], w2hv)
            ones = cpool.tile([128, 1], f32)
            nc.vector.memset(ones[:, :], 1.0)
            # x-edge zero mask for the table build: kills dx'==0 slots on
            # p%32==0 rows (x==0) and dx'==2 slots on p%32==31 rows (x==31).
            mx = cpool.tile([128, 512], f32)
            nc.vector.memset(mx[:, :], 1.0)
            mxv = mx[:, 0:432].rearrange("p (g d c) -> p g d c", d=3, c=16)
            for p in (0, 32, 64, 96):
                nc.vector.memset(mxv[p:p + 1, :, 0, :], 0.0)
            zrow = cpool.tile([1, 144], f32)
            nc.vector.memset(zrow[:, :], 0.0)
            zrv = zrow[0:1, :].rearrange("o (g c) -> o g c", c=16)
            for p in (31, 63, 95, 127):
                # engines can't address partition base 31; DMA can.
                nc.sync.dma_start(mxv[p:p + 1, :, 2, :], zrv)
            # (dma_gather needs the 'mlp' Q7 library; Bacc.finalize inserts
            #  the ModifyPoolConfig loads automatically)
            w1 = cpool.tile([4, 256], f16)
            nc.sync.dma_start(w1[:, :], w1v)
            wk = {}
            for nm in ("w2", "w3", "w4", "w5"):
                N = wkv[nm].shape[1]
                for k in range(2):
                    t = cpool.tile([128, N], f16, tag=f"{nm}_{k}")
                    nc.sync.dma_start(t[:, :], wkv[nm][k * 128:(k + 1) * 128, :])
                    wk[(nm, k)] = t
            bt = {}
            for nm in ("b1", "b2", "b3", "b4"):
                for m in range(2):
                    t = cpool.tile([128, 1], f32, tag=f"{nm}_{m}")
                    nc.sync.dma_start(t[:, :], bv[nm][m * 128:(m + 1) * 128, :])
                    bt[(nm, m)] = t
            for m in range(4):
                t = cpool.tile([128, 1], f32, tag=f"b5_{m}")
                nc.sync.dma_start(t[:, :], bv["b5"][m * 128:(m + 1) * 128, :])
                bt[("b5", m)] = t

            # ====== Phase A0: im2col x2h from padded volume ======
            on32 = cpool.tile([32, 1024], f16)
            nc.vector.memset(on32[:, :], 1.0)
            nc.sync.dma_start(x2h[125, :, :], on32[:, :])
            for dz in range(5):
                for dy in range(5):
                    for dx in range(5):
                        k = dz * 25 + dy * 5 + dx
                        nc.sync.dma_start(
                            x2h[k, :, :].rearrange("z (y x) -> z y x", x=32),
                            p36v[dz:dz + 32, dy:dy + 32, dx:dx + 32],
                        )

            # ================= Phase A: table build =================
            with (
                tc.tile_pool(name="tabsb", bufs=3) as tpool,
                tc.tile_pool(name="tabps", bufs=2, space="PSUM") as tps,
            ):
                def emit_z(z, zlit):
                    # z: int or loop ScalarValue; zlit: 0/31 for the peeled
                    # edge slices, None for the For_i interior body.
                    x2z = tpool.tile([126, 1, 1024], f16, tag="x2z")
                    nc.sync.dma_start(x2z[:, :, :], x2h[:, ds(z, 1), :])
                    tsz = tpool.tile([128, 8, 512], f16, tag="tsz")
                    for blk in range(8):
                        ps = tps.tile([128, 512], f32, tag="tab")
                        nc.tensor.matmul(
                            ps[:, :], x2z[:, 0, blk * 128:(blk + 1) * 128], w2h[:, :],
                            start=True, stop=True,
                        )
                        ts = tsz[:, blk, :]
                        # unfold zero-padding: x-edges via mask-multiply evac,
                        # y-edges via legal-base partition memsets (blk 0/7),
                        # z-edges via channel-range memsets.
                        nc.vector.tensor_tensor(ts[:, :], ps[:, :], mx[:, :], OP.mult)
                        if blk == 0:
                            tv = tsz[0:32, blk, 0:432].rearrange("p (a b) -> p a b", b=144)
                            nc.vector.memset(tv[:, :, 0:48], 0.0)
                        elif blk == 7:
                            tv = tsz[96:128, blk, 0:432].rearrange("p (a b) -> p a b", b=144)
                            nc.vector.memset(tv[:, :, 96:144], 0.0)
                        if zlit == 0:
                            nc.vector.memset(ts[:, 0:144], 0.0)
                        if zlit == 31:
                            nc.vector.memset(ts[:, 288:432], 0.0)
                    # one batched 1MB plain-slice write per z (SP issue-bound)
                    nc.sync.dma_start(table[ds(z, 1), :, :, :], tsz[:, :, :])

                emit_z(0, 0)
                with tc.For_i(1, 31, 1) as zi:
                    emit_z(zi, None)
                emit_z(31, 31)

            if phase == "table":
                with tc.tile_pool(name="dbg", bufs=2) as dbg:
                    for s in range(qpad // 512):
                        t = dbg.tile([1, 512], f16, tag="dbg")
                        tf = dbg.tile([1, 512], f16, tag="dbgh")
                        nc.sync.dma_start(tf[0:1, :], tabflat[s * 37:s * 37 + 1, :])
                        nc.vector.tensor_copy(t[0:1, :], tf[0:1, :])
                        nc.sync.dma_start(out_d[s:s + 1, :], t[0:1, :])

            # ================= Phase B: queries =================
            if phase != "table":
                with (
                    tc.tile_pool(name="mth", bufs=2) as mpool,      # per-macro math
                    tc.tile_pool(name="qf", bufs=10) as qpool,
                    tc.tile_pool(name="mlp", bufs=6) as hpool,      # h sbuf tiles
                    tc.tile_pool(name="pred", bufs=3) as ppool,
                    tc.tile_pool(name="prod", bufs=2) as prpool,
                    tc.tile_pool(name="osb", bufs=3) as opool,
                    tc.tile_pool(name="ps_s", bufs=2, space="PSUM") as ps_small,
                    tc.tile_pool(name="ps_h", bufs=2, space="PSUM") as ps_h,
                    tc.tile_pool(name="ps_p", bufs=2, space="PSUM") as ps_p,
                ):
                    pend = []   # software-pipelined pending dot

                    def emit_dot(ent):
                        qf_s, t, pr_all = ent[:3]
                        osb_m, om = ent[3], ent[4]
                        osum = ps_small.tile([1, 512], f32, tag="osum")
                        prod = prpool.tile([128, 4, 512], f32, tag="prod")
                        nc.vector.tensor_tensor(
                            prod[:, :, :], qf_s[:, :, :], pr_all[:, :, :], OP.mult,
                        )
                        for m in range(4):
                            nc.tensor.matmul(
                                osum[:, :], ones[:, :], prod[:, m, :],
                                start=(m == 0), stop=(m == 3),
                            )
                        nc.scalar.activation(osb_m[0:1, t, :], osum[:, :], AF.Copy)
                        if t == nsub - 1:
                            # one batched output DMA per macro
                            nc.sync.dma_start(out_d[ds(om, 1), :, :, :], osb_m[:, :, :])

                    with tc.For_i(0, nmacro, 1) as mi:
                        # ---- load host-prepped gather indices + MLP inputs ----
                        # replicate the 16 unique idx rows into the 8 groups
                        # the transpose-mode gather expects (DMA writes may
                        # target any partition base, unlike engines)
                        idxr = mpool.tile([128, cols * 8], i16, tag="idxr")
                        for g in range(8):
                            nc.sync.dma_start(idxr[g * 16:(g + 1) * 16, :],
                                              idx_d[ds(mi, 1), :, :])
                        xall = mpool.tile([4, macro], f16, tag="xall")
                        nc.sync.dma_start(xall[:, :], xin_d[ds(mi, 1), :, :])

                        osb_m = opool.tile([1, nsub, 512], f16, tag="osb")

                        # ---- gather q_feat^T (channel-major), one 512-idx
                        # gather per sub-tile (wrapped idx cols contiguous) ----
                        qf_subs = []
                        for s in range(nsub):
                            qf_s = qpool.tile([128, 4, 512], f16, tag="qf")
                            nc.gpsimd.dma_gather(
                                qf_s[:, :, :], tabflat,
                                idxr[:, s * 32:(s + 1) * 32],
                                num_idxs=512, num_idxs_reg=512, elem_size=512,
                                transpose=True,
                            )
                            qf_subs.append(qf_s)

                        # ---- per sub-tile MLP + pipelined dot ----
                        for t in range(nsub):
                            xsb = xall[:, t * 512:(t + 1) * 512]

                            # L1
                            hs = []
                            for m in range(2):
                                ph = ps_h.tile([128, 512], f32, tag="ph")
                                nc.tensor.matmul(ph[:, :], w1[:, m * 128:(m + 1) * 128],
                                                 xsb[:, :], start=True, stop=True)
                                h = hpool.tile([128, 512], f16, tag="h")
                                if m == 0:
                                    nc.scalar.activation(h[:, :], ph[:, :], AF.Relu,
                                                         bias=bt[("b1", m)][:, :])
                                else:
                                    nc.vector.tensor_scalar(h[:, :], ph[:, :],
                                                            bt[("b1", m)][:, :], 0.0,
                                                            OP.add, OP.max)
                                hs.append(h)
                            # L2..L4
                            for li, nm in ((2, "w2"), (3, "w3"), (4, "w4")):
                                nhs = []
                                for m in range(2):
                                    ph = ps_h.tile([128, 512], f32, tag="ph")
                                    nc.tensor.matmul(ph[:, :], wk[(nm, 0)][:, m * 128:(m + 1) * 128],
                                                     hs[0][:, :], start=True, stop=False)
                                    nc.tensor.matmul(ph[:, :], wk[(nm, 1)][:, m * 128:(m + 1) * 128],
                                                     hs[1][:, :], start=False, stop=True)
                                    h = hpool.tile([128, 512], f16, tag="h")
                                    bap = bt[(f"b{li}", m)][:, :]
                                    if m == 0:
                                        nc.scalar.activation(h[:, :], ph[:, :], AF.Relu, bias=bap)
                                    else:
                                        nc.vector.tensor_scalar(h[:, :], ph[:, :], bap, 0.0,
                                                                OP.add, OP.max)
                                    nhs.append(h)
                                hs = nhs
                            # L5 -> pred f16
                            pr_all = ppool.tile([128, 4, 512], f16, tag="pr")
                            for m in range(4):
                                pp = ps_p.tile([128, 512], f32, tag="pp")
                                nc.tensor.matmul(pp[:, :], wk[("w5", 0)][:, m * 128:(m + 1) * 128],
                                                 hs[0][:, :], start=True, stop=False)
                                nc.tensor.matmul(pp[:, :], wk[("w5", 1)][:, m * 128:(m + 1) * 128],
                                                 hs[1][:, :], start=False, stop=True)
                                nc.scalar.activation(pr_all[:, m, :], pp[:, :], AF.Identity,
                                                     bias=bt[("b5", m)][:, :])

                            pend.append((qf_subs[t], t, pr_all, osb_m, mi))
                            if len(pend) > 1:
                                emit_dot(pend.pop(0))
                        # flush within the loop body (cross-iteration
                        # pipelining is blocked by the back-edge barrier)
                        while pend:
                            emit_dot(pend.pop(0))
    nc.finalize()
    return nc


def _prep_queries(coord, cell):
    """Voxel/table index + MLP input per query, f32 (reference arithmetic)."""
    cmu = coord - cell * 0.5
    eps = np.float32(1e-6)
    t1 = np.minimum(np.maximum(cmu + eps, np.float32(-1.0 + 1e-6)),
                    np.float32(1.0 - 1e-6))
    u = t1 * np.float32(16.0) + np.float32(15.5)
    ivox = np.round(u).astype(np.int32)          # RNE, matches HW convert
    iz, iy, ix = ivox[:, 0], ivox[:, 1], ivox[:, 2]
    lin = (iz * 1024 + (iy & 3) * 256 + ix * 8 + (iy >> 2)).astype(np.int16)

    # analytic q_coord (feature-center volume incl. the W-axis index shifts)
    upv = cmu * np.float32(16.0) + np.float32(15.5)
    ri = np.round(upv).astype(np.int32)
    rf = ri.astype(np.float32)
    val = np.all(rf >= 0, axis=1).astype(np.float32)
    rf = np.maximum(rf, np.float32(0.0))
    shv = ((rf[:, 2] < 2.0).astype(np.float32)
           + (rf[:, 2] == 3.0).astype(np.float32)) * np.float32(1.0 / 32.0)
    qcv = rf * np.float32(1.0 / 16.0) - np.float32(31.0 / 32.0)
    qcv = (qcv - shv[:, None]) * val[:, None]
    rel = (cmu - qcv) * np.float32(32.0)
    xin = np.concatenate([rel, cell[:, 0:1] * np.float32(16.0)], axis=1)
    return lin, xin.astype(np.float16)


def _prep_in_maps(inputs, qpad=QPAD, ncores=NCORES, qpc=None):
    coord = np.asarray(inputs["coord"], np.float32)[0]
    cell = np.asarray(inputs["cell"], np.float32)[0]
    blob16, b5p = _build_host_consts(
        inputs["inp"], inputs["W_enc"], inputs["b_enc"],
        inputs["W1"], inputs["W2"], inputs["W3"], inputs["W4"],
        inputs["W5"], inputs["b5"])
    bia = np.concatenate([
        np.asarray(inputs["b1"], np.float32).ravel(),
        np.asarray(inputs["b2"], np.float32).ravel(),
        np.asarray(inputs["b3"], np.float32).ravel(),
        np.asarray(inputs["b4"], np.float32).ravel(),
        b5p.ravel(),
    ]).astype(np.float32)
    assert bia.size == NBIA
    lin, xin = _prep_queries(coord, cell)
    if qpc is None:
        qpc = QTOT // ncores
    pad = qpad - qpc
    in_maps = []
    for c in range(ncores):
        lc = lin[c * qpc:(c + 1) * qpc]
        xc = xin[c * qpc:(c + 1) * qpc]
        lc = np.concatenate([lc, np.repeat(lc[-1:], pad, 0)], 0)
        xc = np.concatenate([xc, np.repeat(xc[-1:], pad, 0)], 0)
        # wrapped gather-index layout: idx[g*16+r, c*8+t] = lin[c*128+t*16+r]
        linm = lc.reshape(NMACRO, COLS, 8, 16)
        idxw = np.transpose(linm, (0, 3, 1, 2)).reshape(NMACRO, 16, COLS * 8)
        xinT = np.ascontiguousarray(
            xc.reshape(NMACRO, MACRO, 4).transpose(0, 2, 1))  # [nmacro,4,macro]
        in_maps.append({"b16p": np.ascontiguousarray(blob16[c * N16P:(c + 1) * N16P]),
                        "idx": np.ascontiguousarray(idxw),
                        "xin": xinT, "bia": bia})
    return in_maps


def kernel(**inputs):
    from concourse import bass_utils

    key = "full"
    if key not in _COMPILED:
        _COMPILED[key] = build_nc()
    nc = _COMPILED[key]
    in_maps = _prep_in_maps(inputs)
    res = bass_utils.run_bass_kernel_spmd(nc, in_maps, core_ids=list(range(NCORES)))
    outs = res.results
    qpc = QTOT // NCORES
    parts = [outs[c]["out"].reshape(-1)[:qpc] for c in range(NCORES)]
    return np.concatenate(parts).reshape(1, QTOT, 1).astype(np.float32)


# revision 24
# speedup vs baseline: 1.0039x; 1.0039x over previous
"""MetaSR (nn_MetaSR_74517682585959) Trainium2 Bass kernel.

Strategy (8 NeuronCores, query-parallel):
 - Replicate encoder+MLP params + feature volume on every core; shard the
   200k queries 8 ways (25000 + pad -> 25088 = 49*512 per core).
 - The metric (warm dispatch wall time over the axon tunnel) is dominated
   by per-call host work, not device compute (~2 ms), so the design
   minimizes (a) input upload bytes, (b) BIR instruction count (walrus
   recompiles on every dispatch).
   a. Inputs: one f16 const blob (padded 36^3 volume + all weights,
      880 KB) uploaded as 1/8th shards and AllGathered on-device, plus
      per-core host-prepped query data in exact device layouts: wrapped
      gather indices (i16, unique 16 rows/macro, replicated to the
      gather's 128-row format on-device) and feature-major MLP inputs
      [rel*32 (3), cell_x*16] (f16). Output f16. ~0.37 MB/core/dispatch
      vs 10.9 MB for the im2col-upload baseline. The voxel-index /
      rel-coord math runs on the host in f32 (bit-identical to the
      reference arithmetic; validated end-to-end in numpy).
   b. For_i hardware loops over the 7 query macro-tiles and the 30
      interior z-slices: ~1.1k BIR instructions vs 5.8k unrolled.
 - Per-core flow:
   1. Build the 5x5x5 im2col x2h[126, 32, 1024] f16 in DRAM from the
      padded volume via 125 strided window DMAs (row 125 = ones row for
      the conv bias). Voxel order within a z-slice is natural col=y*32+x.
   2. Build the unfolded-feature table T[32768 voxels, 512 ch] f16 in
      DRAM via one K=126 matmul per 128-voxel block (conv3x3 o unfold3x3
      collapsed, bias via the ones row). Unfold zero-padding: x-edges via
      a mask-multiply PSUM evacuation (p%32==0 / p%32==31 rows), y-edges
      via partition-range memsets on blk 0/7, z-edges via channel-range
      memsets. The mask is built on-chip (no upload).
   3. Per 3584-query macro-tile: gather q_feat^T via transpose-mode
      dma_gather (f16, channel-major out [128, 4, 512]), one 512-idx
      gather per sub-tile.
   4. MLP 4->256->256->256->256->512(=permuted padded 432) in f16 on PE
      (f32 psum); ReLU+bias fused into PSUM evacuation (ACT/DVE).
   5. out[q] = sum_ch qf*pred: one fused f16xf16->f32 product (DVE),
      then partition-reduce with a ones-vector matmul on PE, software-
      pipelined one sub-tile behind the MLP within each macro body.
"""

import numpy as np

QTOT = 200000
NCORES = 8
QPC = QTOT // NCORES          # 25000
QPAD = 25088                  # 49 * 512
MACRO = 3584                  # 28 * 128 queries per macro tile
NMACRO = QPAD // MACRO        # 7
NSUB = MACRO // 512           # 7
COLS = MACRO // 128           # 28

# f16 blob layout (element offsets)
OFF_P36 = 0
N_P36 = 36 * 36 * 36          # 46656
OFF_W2H = OFF_P36 + N_P36
N_W2H = 126 * 512
OFF_W1 = OFF_W2H + N_W2H
OFF_W2 = OFF_W1 + 4 * 256
OFF_W3 = OFF_W2 + 256 * 256
OFF_W4 = OFF_W3 + 256 * 256
OFF_W5 = OFF_W4 + 256 * 256
N16 = OFF_W5 + 256 * 512      # 439872

# f32 bias blob layout (element offsets)
OFF_B1 = 0
OFF_B2 = OFF_B1 + 256
OFF_B3 = OFF_B2 + 256
OFF_B4 = OFF_B3 + 256
OFF_B5 = OFF_B4 + 256
NBIA = OFF_B5 + 512           # 1536

N16P = N16 // 8               # 54984: per-core AllGather shard of blob16

_COMPILED = {}


def _build_host_consts(inp, W_enc, b_enc, W1, W2, W3, W4, W5, b5):
    """Padded volume, fused conv+unfold weights, f16 MLP weights, w5 perm."""
    v = np.asarray(inp, np.float32)[0, 0]                     # [32,32,32]
    p36 = np.pad(v, 2).astype(np.float16)                     # [36,36,36]

    We = np.asarray(W_enc, np.float32)                        # [16,1,3,3,3]
    w2h = np.zeros((5, 5, 5, 27, 16), np.float32)
    for dz in range(3):
        for dy in range(3):
            for dx in range(3):
                j = dz * 9 + dy * 3 + dx
                for az in range(3):
                    for ay in range(3):
                        for ax in range(3):
                            w2h[dz + az, dy + ay, dx + ax, j, :] = We[:, 0, az, ay, ax]
    w2h_full = np.zeros((126, 512), np.float32)
    w2h_full[:125, :432] = w2h.reshape(125, 432)
    w2h_full[125, :432] = np.tile(np.asarray(b_enc, np.float32), 27)  # j-major bias
    w2h_full = w2h_full.astype(np.float16)

    perm = np.array([c * 27 + j for j in range(27) for c in range(16)], np.int64)
    w5p = np.zeros((256, 512), np.float32)
    w5p[:, :432] = np.asarray(W5, np.float32)[:, perm]
    b5p = np.zeros((512,), np.float32)
    b5p[:432] = np.asarray(b5, np.float32)[perm]

    blob16 = np.concatenate([
        p36.ravel(),
        w2h_full.ravel(),
        np.asarray(W1, np.float32).astype(np.float16).ravel(),
        np.asarray(W2, np.float32).astype(np.float16).ravel(),
        np.asarray(W3, np.float32).astype(np.float16).ravel(),
        np.asarray(W4, np.float32).astype(np.float16).ravel(),
        w5p.astype(np.float16).ravel(),
    ])
    assert blob16.size == N16
    return blob16, b5p


def _patch_tile_drain():
    """Walrus in this toolchain rejects >2 sem waits on the Tile tail drain;
    split the waits across multiple drain instructions."""
    import concourse.mybir as mybir
    from concourse import tile
    from concourse.vector_clock import ScopedClock

    if getattr(tile.TileContext, "_drain_split_patch", False):
        return

    def _drain_and_barrier(self, tick_clock, wait_clock):
        nc = self.nc
        drain_inst = nc.sync.drain()
        wait_clock.add_sem_waits(
            drain_inst.ins, ScopedClock({None: tick_clock.global_clock})
        )
        si = drain_inst.ins.sync_info
        waits = list(si.on_wait) if si is not None else []
        if len(waits) > 1:
            drain_inst.ins.sync_info = mybir.SyncInfo(
                on_wait=waits[:1], on_update=list(si.on_update)
            )
            for w in waits[1:]:
                d2 = nc.sync.drain()
                d2.ins.sync_info = mybir.SyncInfo(on_wait=[w], on_update=[])

    tile.TileContext._drain_and_barrier = _drain_and_barrier
    tile.TileContext._drain_split_patch = True


def build_nc(qpad=QPAD, nmacro=NMACRO, phase="full"):
    import concourse.bass as bass
    import concourse.bacc as bacc
    import concourse.mybir as mybir
    from concourse import tile
    from concourse.bass import ds

    _patch_tile_drain()

    f32 = mybir.dt.float32
    f16 = mybir.dt.float16
    i32 = mybir.dt.int32
    i16 = mybir.dt.int16
    AF = mybir.ActivationFunctionType
    OP = mybir.AluOpType

    macro = MACRO
    nsub = NSUB
    cols = COLS
    assert qpad == nmacro * macro

    nc = bacc.Bacc(None, target_bir_lowering=False, num_devices=NCORES)
    b16p_d = nc.dram_tensor("b16p", [N16P], f16, kind="ExternalInput")
    # per-core query data, host-prepped in device layouts:
    #   idx: wrapped gather indices (the exact idxr SBUF layout per macro)
    #   xin: feature-major MLP inputs [rel*32 (3), cell_x*16] per macro
    idx_d = nc.dram_tensor("idx", [nmacro, 16, cols * 8], i16, kind="ExternalInput")
    xin_d = nc.dram_tensor("xin", [nmacro, 4, macro], f16, kind="ExternalInput")
    bia_d = nc.dram_tensor("bia", [NBIA], f32, kind="ExternalInput")
    out_d = nc.dram_tensor("out", [nmacro, 1, nsub, 512], f16, kind="ExternalOutput")

    bv = {
        "b1": bia_d[OFF_B1:OFF_B2].rearrange("(p o) -> p o", o=1),
        "b2": bia_d[OFF_B2:OFF_B3].rearrange("(p o) -> p o", o=1),
        "b3": bia_d[OFF_B3:OFF_B4].rearrange("(p o) -> p o", o=1),
        "b4": bia_d[OFF_B4:OFF_B5].rearrange("(p o) -> p o", o=1),
        "b5": bia_d[OFF_B5:NBIA].rearrange("(p o) -> p o", o=1),
    }

    with tile.TileContext(nc) as tc:
        with (
            tc.tile_pool(name="dram", bufs=1, space="DRAM") as dpool,
            tc.tile_pool(name="const", bufs=1) as cpool,
        ):
            table = dpool.tile([32, 128, 8, 512], f16)
            tabflat = table[:, :, :, :].rearrange("z p a f -> (z p a) f")
            x2h = dpool.tile([126, 32, 1024], f16)

            # ---- AllGather the replicated const blob (each core uploads
            # 1/8th; collectives can't read I/O tensors, so bounce first) ----
            b16b = dpool.tile([N16P], f16)
            nc.sync.dma_start(b16b[:], b16p_d[:])
            b16g = dpool.tile([N16], f16)
            nc.gpsimd.collective_compute(
                "AllGather", mybir.AluOpType.bypass,
                replica_groups=[list(range(NCORES))],
                ins=[b16b[:]], outs=[b16g[:]],
            )
            p36v = b16g[OFF_P36:OFF_P36 + N_P36].rearrange("(a b c) -> a b c", b=36, c=36)
            w2hv = b16g[OFF_W2H:OFF_W2H + N_W2H].rearrange("(p f) -> p f", f=512)
            w1v = b16g[OFF_W1:OFF_W2].rearrange("(p f) -> p f", f=256)
            wkv = {
                "w2": b16g[OFF_W2:OFF_W3].rearrange("(p f) -> p f", f=256),
                "w3": b16g[OFF_W3:OFF_W4].rearrange("(p f) -> p f", f=256),
                "w4": b16g[OFF_W4:OFF_W5].rearrange("(p f) -> p f", f=256),
                "w5": b16g[OFF_W5:N16].rearrange("(p f) -> p f", f=512),
            }

            # ---- persistent constants in SBUF ----
            w2h = cpool.tile([126, 512], f16)
            nc.sync.dma_start(w2h[:, :], w2hv)
            ones = cpool.tile([128, 1], f32)
            nc.vector.memset(ones[:, :], 1.0)
            # x-edge zero mask for the table build: kills dx'==0 slots on
            # p%32==0 rows (x==0) and dx'==2 slots on p%32==31 rows (x==31).
            mx = cpool.tile([128, 512], f32)
            nc.vector.memset(mx[:, :], 1.0)
            mxv = mx[:, 0:432].rearrange("p (g d c) -> p g d c", d=3, c=16)
            for p in (0, 32, 64, 96):
                nc.vector.memset(mxv[p:p + 1, :, 0, :], 0.0)
            zrow = cpool.tile([1, 144], f32)
            nc.vector.memset(zrow[:, :], 0.0)
            zrv = zrow[0:1, :].rearrange("o (g c) -> o g c", c=16)
            for p in (31, 63, 95, 127):
                # engines can't address partition base 31; DMA can.
                nc.sync.dma_start(mxv[p:p + 1, :, 2, :], zrv)
            # (dma_gather needs the 'mlp' Q7 library; Bacc.finalize inserts
            #  the ModifyPoolConfig loads automatically)
            w1 = cpool.tile([4, 256], f16)
            nc.sync.dma_start(w1[:, :], w1v)
            wk = {}
            for nm in ("w2", "w3", "w4", "w5"):
                N = wkv[nm].shape[1]
                for k in range(2):
                    t = cpool.tile([128, N], f16, tag=f"{nm}_{k}")
                    nc.sync.dma_start(t[:, :], wkv[nm][k * 128:(k + 1) * 128, :])
                    wk[(nm, k)] = t
            bt = {}
            for nm in ("b1", "b2", "b3", "b4"):
                for m in range(2):
                    t = cpool.tile([128, 1], f32, tag=f"{nm}_{m}")
                    nc.sync.dma_start(t[:, :], bv[nm][m * 128:(m + 1) * 128, :])
                    bt[(nm, m)] = t
            for m in range(4):
                t = cpool.tile([128, 1], f32, tag=f"b5_{m}")
                nc.sync.dma_start(t[:, :], bv["b5"][m * 128:(m + 1) * 128, :])
                bt[("b5", m)] = t

            # ====== Phase A0: im2col x2h from padded volume ======
            on32 = cpool.tile([32, 1024], f16)
            nc.vector.memset(on32[:, :], 1.0)
            nc.sync.dma_start(x2h[125, :, :], on32[:, :])
            for dz in range(5):
                for dy in range(5):
                    for dx in range(5):
                        k = dz * 25 + dy * 5 + dx
                        nc.sync.dma_start(
                            x2h[k, :, :].rearrange("z (y x) -> z y x", x=32),
                            p36v[dz:dz + 32, dy:dy + 32, dx:dx + 32],
                        )

            # ================= Phase A: table build =================
            with (
                tc.tile_pool(name="tabsb", bufs=3) as tpool,
                tc.tile_pool(name="tabps", bufs=2, space="PSUM") as tps,
            ):
                def emit_z(z, zlit):
                    # z: int or loop ScalarValue; zlit: 0/31 for the peeled
                    # edge slices, None for the For_i interior body.
                    x2z = tpool.tile([126, 1, 1024], f16, tag="x2z")
                    nc.sync.dma_start(x2z[:, :, :], x2h[:, ds(z, 1), :])
                    tsz = tpool.tile([128, 8, 512], f16, tag="tsz")
                    for blk in range(8):
                        ps = tps.tile([128, 512], f32, tag="tab")
                        nc.tensor.matmul(
                            ps[:, :], x2z[:, 0, blk * 128:(blk + 1) * 128], w2h[:, :],
                            start=True, stop=True,
                        )
                        ts = tsz[:, blk, :]
                        # unfold zero-padding: x-edges via mask-multiply evac,
                        # y-edges via legal-base partition memsets (blk 0/7),
                        # z-edges via channel-range memsets.
                        nc.vector.tensor_tensor(ts[:, :], ps[:, :], mx[:, :], OP.mult)
                        if blk == 0:
                            tv = tsz[0:32, blk, 0:432].rearrange("p (a b) -> p a b", b=144)
                            nc.vector.memset(tv[:, :, 0:48], 0.0)
                        elif blk == 7:
                            tv = tsz[96:128, blk, 0:432].rearrange("p (a b) -> p a b", b=144)
                            nc.vector.memset(tv[:, :, 96:144], 0.0)
                        if zlit == 0:
                            nc.vector.memset(ts[:, 0:144], 0.0)
                        if zlit == 31:
                            nc.vector.memset(ts[:, 288:432], 0.0)
                    # one batched 1MB plain-slice write per z (SP issue-bound)
                    nc.sync.dma_start(table[ds(z, 1), :, :, :], tsz[:, :, :])

                emit_z(0, 0)
                with tc.For_i(1, 31, 1) as zi:
                    emit_z(zi, None)
                emit_z(31, 31)

            if phase == "table":
                with tc.tile_pool(name="dbg", bufs=2) as dbg:
                    for s in range(qpad // 512):
                        t = dbg.tile([1, 512], f16, tag="dbg")
                        tf = dbg.tile([1, 512], f16, tag="dbgh")
                        nc.sync.dma_start(tf[0:1, :], tabflat[s * 37:s * 37 + 1, :])
                        nc.vector.tensor_copy(t[0:1, :], tf[0:1, :])
                        nc.sync.dma_start(out_d[s:s + 1, :], t[0:1, :])

            # ================= Phase B: queries =================
            if phase != "table":
                with (
                    tc.tile_pool(name="mth", bufs=2) as mpool,      # per-macro math
                    tc.tile_pool(name="qf", bufs=10) as qpool,
                    tc.tile_pool(name="mlp", bufs=6) as hpool,      # h sbuf tiles
                    tc.tile_pool(name="pred", bufs=3) as ppool,
                    tc.tile_pool(name="prod", bufs=2) as prpool,
                    tc.tile_pool(name="osb", bufs=3) as opool,
                    tc.tile_pool(name="ps_s", bufs=2, space="PSUM") as ps_small,
                    tc.tile_pool(name="ps_h", bufs=2, space="PSUM") as ps_h,
                    tc.tile_pool(name="ps_p", bufs=2, space="PSUM") as ps_p,
                ):
                    pend = []   # software-pipelined pending dot

                    def emit_dot(ent):
                        qf_s, t, pr_all = ent[:3]
                        osb_m, om = ent[3], ent[4]
                        osum = ps_small.tile([1, 512], f32, tag="osum")
                        prod = prpool.tile([128, 4, 512], f32, tag="prod")
                        nc.vector.tensor_tensor(
                            prod[:, :, :], qf_s[:, :, :], pr_all[:, :, :], OP.mult,
                        )
                        for m in range(4):
                            nc.tensor.matmul(
                                osum[:, :], ones[:, :], prod[:, m, :],
                                start=(m == 0), stop=(m == 3),
                            )
                        nc.scalar.activation(osb_m[0:1, t, :], osum[:, :], AF.Copy)
                        if t == nsub - 1:
                            # one batched output DMA per macro
                            nc.sync.dma_start(out_d[ds(om, 1), :, :, :], osb_m[:, :, :])

                    with tc.For_i(0, nmacro, 1) as mi:
                        # ---- load host-prepped gather indices + MLP inputs ----
                        # replicate the 16 unique idx rows into the 8 groups
                        # the transpose-mode gather expects (DMA writes may
                        # target any partition base, unlike engines)
                        idxr = mpool.tile([128, cols * 8], i16, tag="idxr")
                        for g in range(8):
                            nc.sync.dma_start(idxr[g * 16:(g + 1) * 16, :],
                                              idx_d[ds(mi, 1), :, :])
                        xall = mpool.tile([4, macro], f16, tag="xall")
                        nc.sync.dma_start(xall[:, :], xin_d[ds(mi, 1), :, :])

                        osb_m = opool.tile([1, nsub, 512], f16, tag="osb")

                        # ---- gather q_feat^T (channel-major), one 512-idx
                        # gather per sub-tile (wrapped idx cols contiguous) ----
                        qf_subs = []
                        for s in range(nsub):
                            qf_s = qpool.tile([128, 4, 512], f16, tag="qf")
                            nc.gpsimd.dma_gather(
                                qf_s[:, :, :], tabflat,
                                idxr[:, s * 32:(s + 1) * 32],
                                num_idxs=512, num_idxs_reg=512, elem_size=512,
                                transpose=True,
                            )
                            qf_subs.append(qf_s)

                        # ---- per sub-tile MLP + pipelined dot ----
                        for t in range(nsub):
                            xsb = xall[:, t * 512:(t + 1) * 512]

                            # L1
                            hs = []
                            for m in range(2):
                                ph = ps_h.tile([128, 512], f32, tag="ph")
                                nc.tensor.matmul(ph[:, :], w1[:, m * 128:(m + 1) * 128],
                                                 xsb[:, :], start=True, stop=True)
                                h = hpool.tile([128, 512], f16, tag="h")
                                if m == 0:
                                    nc.scalar.activation(h[:, :], ph[:, :], AF.Relu,
                                                         bias=bt[("b1", m)][:, :])
                                else:
                                    nc.vector.tensor_scalar(h[:, :], ph[:, :],
                                                            bt[("b1", m)][:, :], 0.0,
                                                            OP.add, OP.max)
                                hs.append(h)
                            # L2..L4
                            for li, nm in ((2, "w2"), (3, "w3"), (4, "w4")):
                                nhs = []
                                for m in range(2):
                                    ph = ps_h.tile([128, 512], f32, tag="ph")
                                    nc.tensor.matmul(ph[:, :], wk[(nm, 0)][:, m * 128:(m + 1) * 128],
                                                     hs[0][:, :], start=True, stop=False)
                                    nc.tensor.matmul(ph[:, :], wk[(nm, 1)][:, m * 128:(m + 1) * 128],
                                                     hs[1][:, :], start=False, stop=True)
                                    h = hpool.tile([128, 512], f16, tag="h")
                                    bap = bt[(f"b{li}", m)][:, :]
                                    if m == 0:
                                        nc.scalar.activation(h[:, :], ph[:, :], AF.Relu, bias=bap)
                                    else:
                                        nc.vector.tensor_scalar(h[:, :], ph[:, :], bap, 0.0,
                                                                OP.add, OP.max)
                                    nhs.append(h)
                                hs = nhs
                            # L5 -> pred f16
                            pr_all = ppool.tile([128, 4, 512], f16, tag="pr")
                            for m in range(4):
                                pp = ps_p.tile([128, 512], f32, tag="pp")
                                nc.tensor.matmul(pp[:, :], wk[("w5", 0)][:, m * 128:(m + 1) * 128],
                                                 hs[0][:, :], start=True, stop=False)
                                nc.tensor.matmul(pp[:, :], wk[("w5", 1)][:, m * 128:(m + 1) * 128],
                                                 hs[1][:, :], start=False, stop=True)
                                nc.scalar.activation(pr_all[:, m, :], pp[:, :], AF.Identity,
                                                     bias=bt[("b5", m)][:, :])

                            pend.append((qf_subs[t], t, pr_all, osb_m, mi))
                            if len(pend) > 1:
                                emit_dot(pend.pop(0))
                        # flush within the loop body (cross-iteration
                        # pipelining is blocked by the back-edge barrier)
                        while pend:
                            emit_dot(pend.pop(0))
    nc.finalize()
    return nc


def _prep_queries(coord, cell):
    """Voxel/table index + MLP input per query, f32 (reference arithmetic)."""
    cmu = coord - cell * 0.5
    eps = np.float32(1e-6)
    t1 = np.minimum(np.maximum(cmu + eps, np.float32(-1.0 + 1e-6)),
                    np.float32(1.0 - 1e-6))
    u = t1 * np.float32(16.0) + np.float32(15.5)
    ivox = np.round(u).astype(np.int32)          # RNE, matches HW convert
    iz, iy, ix = ivox[:, 0], ivox[:, 1], ivox[:, 2]
    lin = (iz * 1024 + (iy & 3) * 256 + ix * 8 + (iy >> 2)).astype(np.int16)

    # analytic q_coord (feature-center volume incl. the W-axis index shifts)
    upv = cmu * np.float32(16.0) + np.float32(15.5)
    ri = np.round(upv).astype(np.int32)
    rf = ri.astype(np.float32)
    val = np.all(rf >= 0, axis=1).astype(np.float32)
    rf = np.maximum(rf, np.float32(0.0))
    shv = ((rf[:, 2] < 2.0).astype(np.float32)
           + (rf[:, 2] == 3.0).astype(np.float32)) * np.float32(1.0 / 32.0)
    qcv = rf * np.float32(1.0 / 16.0) - np.float32(31.0 / 32.0)
    qcv = (qcv - shv[:, None]) * val[:, None]
    rel = (cmu - qcv) * np.float32(32.0)
    xin = np.concatenate([rel, cell[:, 0:1] * np.float32(16.0)], axis=1)
    return lin, xin.astype(np.float16)


def _prep_in_maps(inputs, qpad=QPAD, ncores=NCORES, qpc=None):
    coord = np.asarray(inputs["coord"], np.float32)[0]
    cell = np.asarray(inputs["cell"], np.float32)[0]
    blob16, b5p = _build_host_consts(
        inputs["inp"], inputs["W_enc"], inputs["b_enc"],
        inputs["W1"], inputs["W2"], inputs["W3"], inputs["W4"],
        inputs["W5"], inputs["b5"])
    bia = np.concatenate([
        np.asarray(inputs["b1"], np.float32).ravel(),
        np.asarray(inputs["b2"], np.float32).ravel(),
        np.asarray(inputs["b3"], np.float32).ravel(),
        np.asarray(inputs["b4"], np.float32).ravel(),
        b5p.ravel(),
    ]).astype(np.float32)
    assert bia.size == NBIA
    lin, xin = _prep_queries(coord, cell)
    if qpc is None:
        qpc = QTOT // ncores
    pad = qpad - qpc
    in_maps = []
    for c in range(ncores):
        lc = lin[c * qpc:(c + 1) * qpc]
        xc = xin[c * qpc:(c + 1) * qpc]
        lc = np.concatenate([lc, np.repeat(lc[-1:], pad, 0)], 0)
        xc = np.concatenate([xc, np.repeat(xc[-1:], pad, 0)], 0)
        # wrapped gather-index layout: idx[g*16+r, c*8+t] = lin[c*128+t*16+r]
        linm = lc.reshape(NMACRO, COLS, 8, 16)
        idxw = np.transpose(linm, (0, 3, 1, 2)).reshape(NMACRO, 16, COLS * 8)
        xinT = np.ascontiguousarray(
            xc.reshape(NMACRO, MACRO, 4).transpose(0, 2, 1))  # [nmacro,4,macro]
        in_maps.append({"b16p": np.ascontiguousarray(blob16[c * N16P:(c + 1) * N16P]),
                        "idx": np.ascontiguousarray(idxw),
                        "xin": xinT, "bia": bia})
    return in_maps


def kernel(**inputs):
    from concourse import bass_utils

    key = "full"
    if key not in _COMPILED:
        _COMPILED[key] = build_nc()
    nc = _COMPILED[key]
    in_maps = _prep_in_maps(inputs)
    res = bass_utils.run_bass_kernel_spmd(nc, in_maps, core_ids=list(range(NCORES)))
    outs = res.results
    qpc = QTOT // NCORES
    parts = [outs[c]["out"].reshape(-1)[:qpc] for c in range(NCORES)]
    return np.concatenate(parts).reshape(1, QTOT, 1).astype(np.float32)


# revision 25
# speedup vs baseline: 2.5766x; 2.5667x over previous
"""MetaSR (nn_MetaSR_74517682585959) Trainium2 Bass kernel.

Strategy (8 NeuronCores, query-parallel):
 - Replicate encoder+MLP params + feature volume on every core; shard the
   200k queries 8 ways (25000 + pad -> 25088 = 49*512 per core).
 - The metric (warm dispatch wall time over the axon tunnel) is dominated
   by per-call host work, not device compute (~2 ms), so the design
   minimizes (a) input upload bytes, (b) BIR instruction count (walrus
   recompiles on every dispatch).
   a. Inputs: one f16 const blob (padded 36^3 volume + all weights,
      880 KB) uploaded as 1/8th shards and AllGathered on-device, plus
      per-core host-prepped query data in exact device layouts: wrapped
      gather indices (i16, unique 16 rows/macro, replicated to the
      gather's 128-row format on-device) and feature-major MLP inputs
      [rel*32 (3), cell_x*16] (f16). Output f16. ~0.37 MB/core/dispatch
      vs 10.9 MB for the im2col-upload baseline. The voxel-index /
      rel-coord math runs on the host in f32 (bit-identical to the
      reference arithmetic; validated end-to-end in numpy).
   b. For_i hardware loops over the 7 query macro-tiles and the 30
      interior z-slices: ~1.1k BIR instructions vs 5.8k unrolled.
 - Per-core flow:
   1. Build the 5x5x5 im2col x2h[126, 32, 1024] f16 in DRAM from the
      padded volume via 125 strided window DMAs (row 125 = ones row for
      the conv bias). Voxel order within a z-slice is natural col=y*32+x.
   2. Build the unfolded-feature table T[32768 voxels, 512 ch] f16 in
      DRAM via one K=126 matmul per 128-voxel block (conv3x3 o unfold3x3
      collapsed, bias via the ones row). Unfold zero-padding: x-edges via
      a mask-multiply PSUM evacuation (p%32==0 / p%32==31 rows), y-edges
      via partition-range memsets on blk 0/7, z-edges via channel-range
      memsets. The mask is built on-chip (no upload).
   3. Per 3584-query macro-tile: gather q_feat^T via transpose-mode
      dma_gather (f16, channel-major out [128, 4, 512]), one 512-idx
      gather per sub-tile.
   4. MLP 4->256->256->256->256->512(=permuted padded 432) in f16 on PE
      (f32 psum); ReLU+bias fused into PSUM evacuation (ACT/DVE).
   5. out[q] = sum_ch qf*pred: one fused f16xf16->f32 product (DVE),
      then partition-reduce with a ones-vector matmul on PE, software-
      pipelined one sub-tile behind the MLP within each macro body.
"""

import numpy as np


def _enable_jax_pcc():
    """Enable jax's persistent compilation cache: the axon dispatch path
    rebuilds its jit closure on every call, which otherwise re-runs the
    neuronx/walrus compile (~0.11 s) per dispatch. With the cache, the
    compiled executable is reloaded from disk instead."""
    try:
        import jax
        jax.config.update("jax_compilation_cache_dir", "/tmp/jax_pcc")
        jax.config.update("jax_persistent_cache_min_compile_time_secs", 0.0)
        jax.config.update("jax_persistent_cache_min_entry_size_bytes", 0)
    except Exception:
        pass


_enable_jax_pcc()

QTOT = 200000
NCORES = 8
QPC = QTOT // NCORES          # 25000
QPAD = 25088                  # 49 * 512
MACRO = 3584                  # 28 * 128 queries per macro tile
NMACRO = QPAD // MACRO        # 7
NSUB = MACRO // 512           # 7
COLS = MACRO // 128           # 28

# f16 blob layout (element offsets)
OFF_P36 = 0
N_P36 = 36 * 36 * 36          # 46656
OFF_W2H = OFF_P36 + N_P36
N_W2H = 126 * 512
OFF_W1 = OFF_W2H + N_W2H
OFF_W2 = OFF_W1 + 4 * 256
OFF_W3 = OFF_W2 + 256 * 256
OFF_W4 = OFF_W3 + 256 * 256
OFF_W5 = OFF_W4 + 256 * 256
N16 = OFF_W5 + 256 * 512      # 439872

# f32 bias blob layout (element offsets)
OFF_B1 = 0
OFF_B2 = OFF_B1 + 256
OFF_B3 = OFF_B2 + 256
OFF_B4 = OFF_B3 + 256
OFF_B5 = OFF_B4 + 256
NBIA = OFF_B5 + 512           # 1536

N16P = N16 // 8               # 54984: per-core AllGather shard of blob16

_COMPILED = {}


def _build_host_consts(inp, W_enc, b_enc, W1, W2, W3, W4, W5, b5):
    """Padded volume, fused conv+unfold weights, f16 MLP weights, w5 perm."""
    v = np.asarray(inp, np.float32)[0, 0]                     # [32,32,32]
    p36 = np.pad(v, 2).astype(np.float16)                     # [36,36,36]

    We = np.asarray(W_enc, np.float32)                        # [16,1,3,3,3]
    w2h = np.zeros((5, 5, 5, 27, 16), np.float32)
    for dz in range(3):
        for dy in range(3):
            for dx in range(3):
                j = dz * 9 + dy * 3 + dx
                for az in range(3):
                    for ay in range(3):
                        for ax in range(3):
                            w2h[dz + az, dy + ay, dx + ax, j, :] = We[:, 0, az, ay, ax]
    w2h_full = np.zeros((126, 512), np.float32)
    w2h_full[:125, :432] = w2h.reshape(125, 432)
    w2h_full[125, :432] = np.tile(np.asarray(b_enc, np.float32), 27)  # j-major bias
    w2h_full = w2h_full.astype(np.float16)

    perm = np.array([c * 27 + j for j in range(27) for c in range(16)], np.int64)
    w5p = np.zeros((256, 512), np.float32)
    w5p[:, :432] = np.asarray(W5, np.float32)[:, perm]
    b5p = np.zeros((512,), np.float32)
    b5p[:432] = np.asarray(b5, np.float32)[perm]

    blob16 = np.concatenate([
        p36.ravel(),
        w2h_full.ravel(),
        np.asarray(W1, np.float32).astype(np.float16).ravel(),
        np.asarray(W2, np.float32).astype(np.float16).ravel(),
        np.asarray(W3, np.float32).astype(np.float16).ravel(),
        np.asarray(W4, np.float32).astype(np.float16).ravel(),
        w5p.astype(np.float16).ravel(),
    ])
    assert blob16.size == N16
    return blob16, b5p


def _patch_tile_drain():
    """Walrus in this toolchain rejects >2 sem waits on the Tile tail drain;
    split the waits across multiple drain instructions."""
    import concourse.mybir as mybir
    from concourse import tile
    from concourse.vector_clock import ScopedClock

    if getattr(tile.TileContext, "_drain_split_patch", False):
        return

    def _drain_and_barrier(self, tick_clock, wait_clock):
        nc = self.nc
        drain_inst = nc.sync.drain()
        wait_clock.add_sem_waits(
            drain_inst.ins, ScopedClock({None: tick_clock.global_clock})
        )
        si = drain_inst.ins.sync_info
        waits = list(si.on_wait) if si is not None else []
        if len(waits) > 1:
            drain_inst.ins.sync_info = mybir.SyncInfo(
                on_wait=waits[:1], on_update=list(si.on_update)
            )
            for w in waits[1:]:
                d2 = nc.sync.drain()
                d2.ins.sync_info = mybir.SyncInfo(on_wait=[w], on_update=[])

    tile.TileContext._drain_and_barrier = _drain_and_barrier
    tile.TileContext._drain_split_patch = True


def build_nc(qpad=QPAD, nmacro=NMACRO, phase="full"):
    import concourse.bass as bass
    import concourse.bacc as bacc
    import concourse.mybir as mybir
    from concourse import tile
    from concourse.bass import ds

    _patch_tile_drain()

    f32 = mybir.dt.float32
    f16 = mybir.dt.float16
    i32 = mybir.dt.int32
    i16 = mybir.dt.int16
    AF = mybir.ActivationFunctionType
    OP = mybir.AluOpType

    macro = MACRO
    nsub = NSUB
    cols = COLS
    assert qpad == nmacro * macro

    nc = bacc.Bacc(None, target_bir_lowering=False, num_devices=NCORES)
    b16p_d = nc.dram_tensor("b16p", [N16P], f16, kind="ExternalInput")
    # per-core query data, host-prepped in device layouts:
    #   idx: wrapped gather indices (the exact idxr SBUF layout per macro)
    #   xin: feature-major MLP inputs [rel*32 (3), cell_x*16] per macro
    idx_d = nc.dram_tensor("idx", [nmacro, 16, cols * 8], i16, kind="ExternalInput")
    xin_d = nc.dram_tensor("xin", [nmacro, 4, macro], f16, kind="ExternalInput")
    bia_d = nc.dram_tensor("bia", [NBIA], f32, kind="ExternalInput")
    out_d = nc.dram_tensor("out", [nmacro, 1, nsub, 512], f16, kind="ExternalOutput")

    bv = {
        "b1": bia_d[OFF_B1:OFF_B2].rearrange("(p o) -> p o", o=1),
        "b2": bia_d[OFF_B2:OFF_B3].rearrange("(p o) -> p o", o=1),
        "b3": bia_d[OFF_B3:OFF_B4].rearrange("(p o) -> p o", o=1),
        "b4": bia_d[OFF_B4:OFF_B5].rearrange("(p o) -> p o", o=1),
        "b5": bia_d[OFF_B5:NBIA].rearrange("(p o) -> p o", o=1),
    }

    with tile.TileContext(nc) as tc:
        with (
            tc.tile_pool(name="dram", bufs=1, space="DRAM") as dpool,
            tc.tile_pool(name="const", bufs=1) as cpool,
        ):
            table = dpool.tile([32, 128, 8, 512], f16)
            tabflat = table[:, :, :, :].rearrange("z p a f -> (z p a) f")
            x2h = dpool.tile([126, 32, 1024], f16)

            # ---- AllGather the replicated const blob (each core uploads
            # 1/8th; collectives can't read I/O tensors, so bounce first) ----
            b16b = dpool.tile([N16P], f16)
            nc.sync.dma_start(b16b[:], b16p_d[:])
            b16g = dpool.tile([N16], f16)
            nc.gpsimd.collective_compute(
                "AllGather", mybir.AluOpType.bypass,
                replica_groups=[list(range(NCORES))],
                ins=[b16b[:]], outs=[b16g[:]],
            )
            p36v = b16g[OFF_P36:OFF_P36 + N_P36].rearrange("(a b c) -> a b c", b=36, c=36)
            w2hv = b16g[OFF_W2H:OFF_W2H + N_W2H].rearrange("(p f) -> p f", f=512)
            w1v = b16g[OFF_W1:OFF_W2].rearrange("(p f) -> p f", f=256)
            wkv = {
                "w2": b16g[OFF_W2:OFF_W3].rearrange("(p f) -> p f", f=256),
                "w3": b16g[OFF_W3:OFF_W4].rearrange("(p f) -> p f", f=256),
                "w4": b16g[OFF_W4:OFF_W5].rearrange("(p f) -> p f", f=256),
                "w5": b16g[OFF_W5:N16].rearrange("(p f) -> p f", f=512),
            }

            # ---- persistent constants in SBUF ----
            w2h = cpool.tile([126, 512], f16)
            nc.sync.dma_start(w2h[:, :], w2hv)
            ones = cpool.tile([128, 1], f32)
            nc.vector.memset(ones[:, :], 1.0)
            # x-edge zero mask for the table build: kills dx'==0 slots on
            # p%32==0 rows (x==0) and dx'==2 slots on p%32==31 rows (x==31).
            mx = cpool.tile([128, 512], f32)
            nc.vector.memset(mx[:, :], 1.0)
            mxv = mx[:, 0:432].rearrange("p (g d c) -> p g d c", d=3, c=16)
            for p in (0, 32, 64, 96):
                nc.vector.memset(mxv[p:p + 1, :, 0, :], 0.0)
            zrow = cpool.tile([1, 144], f32)
            nc.vector.memset(zrow[:, :], 0.0)
            zrv = zrow[0:1, :].rearrange("o (g c) -> o g c", c=16)
            for p in (31, 63, 95, 127):
                # engines can't address partition base 31; DMA can.
                nc.sync.dma_start(mxv[p:p + 1, :, 2, :], zrv)
            # (dma_gather needs the 'mlp' Q7 library; Bacc.finalize inserts
            #  the ModifyPoolConfig loads automatically)
            w1 = cpool.tile([4, 256], f16)
            nc.sync.dma_start(w1[:, :], w1v)
            wk = {}
            for nm in ("w2", "w3", "w4", "w5"):
                N = wkv[nm].shape[1]
                for k in range(2):
                    t = cpool.tile([128, N], f16, tag=f"{nm}_{k}")
                    nc.sync.dma_start(t[:, :], wkv[nm][k * 128:(k + 1) * 128, :])
                    wk[(nm, k)] = t
            bt = {}
            for nm in ("b1", "b2", "b3", "b4"):
                for m in range(2):
                    t = cpool.tile([128, 1], f32, tag=f"{nm}_{m}")
                    nc.sync.dma_start(t[:, :], bv[nm][m * 128:(m + 1) * 128, :])
                    bt[(nm, m)] = t
            for m in range(4):
                t = cpool.tile([128, 1], f32, tag=f"b5_{m}")
                nc.sync.dma_start(t[:, :], bv["b5"][m * 128:(m + 1) * 128, :])
                bt[("b5", m)] = t

            # ====== Phase A0: im2col x2h from padded volume ======
            on32 = cpool.tile([32, 1024], f16)
            nc.vector.memset(on32[:, :], 1.0)
            nc.sync.dma_start(x2h[125, :, :], on32[:, :])
            for dz in range(5):
                for dy in range(5):
                    for dx in range(5):
                        k = dz * 25 + dy * 5 + dx
                        nc.sync.dma_start(
                            x2h[k, :, :].rearrange("z (y x) -> z y x", x=32),
                            p36v[dz:dz + 32, dy:dy + 32, dx:dx + 32],
                        )

            # ================= Phase A: table build =================
            with (
                tc.tile_pool(name="tabsb", bufs=3) as tpool,
                tc.tile_pool(name="tabps", bufs=2, space="PSUM") as tps,
            ):
                def emit_z(z, zlit):
                    # z: int or loop ScalarValue; zlit: 0/31 for the peeled
                    # edge slices, None for the For_i interior body.
                    x2z = tpool.tile([126, 1, 1024], f16, tag="x2z")
                    nc.sync.dma_start(x2z[:, :, :], x2h[:, ds(z, 1), :])
                    tsz = tpool.tile([128, 8, 512], f16, tag="tsz")
                    for blk in range(8):
                        ps = tps.tile([128, 512], f32, tag="tab")
                        nc.tensor.matmul(
                            ps[:, :], x2z[:, 0, blk * 128:(blk + 1) * 128], w2h[:, :],
                            start=True, stop=True,
                        )
                        ts = tsz[:, blk, :]
                        # unfold zero-padding: x-edges via mask-multiply evac,
                        # y-edges via legal-base partition memsets (blk 0/7),
                        # z-edges via channel-range memsets.
                        nc.vector.tensor_tensor(ts[:, :], ps[:, :], mx[:, :], OP.mult)
                        if blk == 0:
                            tv = tsz[0:32, blk, 0:432].rearrange("p (a b) -> p a b", b=144)
                            nc.vector.memset(tv[:, :, 0:48], 0.0)
                        elif blk == 7:
                            tv = tsz[96:128, blk, 0:432].rearrange("p (a b) -> p a b", b=144)
                            nc.vector.memset(tv[:, :, 96:144], 0.0)
                        if zlit == 0:
                            nc.vector.memset(ts[:, 0:144], 0.0)
                        if zlit == 31:
                            nc.vector.memset(ts[:, 288:432], 0.0)
                    # one batched 1MB plain-slice write per z (SP issue-bound)
                    nc.sync.dma_start(table[ds(z, 1), :, :, :], tsz[:, :, :])

                emit_z(0, 0)
                with tc.For_i(1, 31, 1) as zi:
                    emit_z(zi, None)
                emit_z(31, 31)

            if phase == "table":
                with tc.tile_pool(name="dbg", bufs=2) as dbg:
                    for s in range(qpad // 512):
                        t = dbg.tile([1, 512], f16, tag="dbg")
                        tf = dbg.tile([1, 512], f16, tag="dbgh")
                        nc.sync.dma_start(tf[0:1, :], tabflat[s * 37:s * 37 + 1, :])
                        nc.vector.tensor_copy(t[0:1, :], tf[0:1, :])
                        nc.sync.dma_start(out_d[s:s + 1, :], t[0:1, :])

            # ================= Phase B: queries =================
            if phase != "table":
                with (
                    tc.tile_pool(name="mth", bufs=2) as mpool,      # per-macro math
                    tc.tile_pool(name="qf", bufs=10) as qpool,
                    tc.tile_pool(name="mlp", bufs=6) as hpool,      # h sbuf tiles
                    tc.tile_pool(name="pred", bufs=3) as ppool,
                    tc.tile_pool(name="prod", bufs=2) as prpool,
                    tc.tile_pool(name="osb", bufs=3) as opool,
                    tc.tile_pool(name="ps_s", bufs=2, space="PSUM") as ps_small,
                    tc.tile_pool(name="ps_h", bufs=2, space="PSUM") as ps_h,
                    tc.tile_pool(name="ps_p", bufs=2, space="PSUM") as ps_p,
                ):
                    pend = []   # software-pipelined pending dot

                    def emit_dot(ent):
                        qf_s, t, pr_all = ent[:3]
                        osb_m, om = ent[3], ent[4]
                        osum = ps_small.tile([1, 512], f32, tag="osum")
                        prod = prpool.tile([128, 4, 512], f32, tag="prod")
                        nc.vector.tensor_tensor(
                            prod[:, :, :], qf_s[:, :, :], pr_all[:, :, :], OP.mult,
                        )
                        for m in range(4):
                            nc.tensor.matmul(
                                osum[:, :], ones[:, :], prod[:, m, :],
                                start=(m == 0), stop=(m == 3),
                            )
                        nc.scalar.activation(osb_m[0:1, t, :], osum[:, :], AF.Copy)
                        if t == nsub - 1:
                            # one batched output DMA per macro
                            nc.sync.dma_start(out_d[ds(om, 1), :, :, :], osb_m[:, :, :])

                    with tc.For_i(0, nmacro, 1) as mi:
                        # ---- load host-prepped gather indices + MLP inputs ----
                        # replicate the 16 unique idx rows into the 8 groups
                        # the transpose-mode gather expects (DMA writes may
                        # target any partition base, unlike engines)
                        idxr = mpool.tile([128, cols * 8], i16, tag="idxr")
                        for g in range(8):
                            nc.sync.dma_start(idxr[g * 16:(g + 1) * 16, :],
                                              idx_d[ds(mi, 1), :, :])
                        xall = mpool.tile([4, macro], f16, tag="xall")
                        nc.sync.dma_start(xall[:, :], xin_d[ds(mi, 1), :, :])

                        osb_m = opool.tile([1, nsub, 512], f16, tag="osb")

                        # ---- gather q_feat^T (channel-major), one 512-idx
                        # gather per sub-tile (wrapped idx cols contiguous) ----
                        qf_subs = []
                        for s in range(nsub):
                            qf_s = qpool.tile([128, 4, 512], f16, tag="qf")
                            nc.gpsimd.dma_gather(
                                qf_s[:, :, :], tabflat,
                                idxr[:, s * 32:(s + 1) * 32],
                                num_idxs=512, num_idxs_reg=512, elem_size=512,
                                transpose=True,
                            )
                            qf_subs.append(qf_s)

                        # ---- per sub-tile MLP + pipelined dot ----
                        for t in range(nsub):
                            xsb = xall[:, t * 512:(t + 1) * 512]

                            # L1
                            hs = []
                            for m in range(2):
                                ph = ps_h.tile([128, 512], f32, tag="ph")
                                nc.tensor.matmul(ph[:, :], w1[:, m * 128:(m + 1) * 128],
                                                 xsb[:, :], start=True, stop=True)
                                h = hpool.tile([128, 512], f16, tag="h")
                                if m == 0:
                                    nc.scalar.activation(h[:, :], ph[:, :], AF.Relu,
                                                         bias=bt[("b1", m)][:, :])
                                else:
                                    nc.vector.tensor_scalar(h[:, :], ph[:, :],
                                                            bt[("b1", m)][:, :], 0.0,
                                                            OP.add, OP.max)
                                hs.append(h)
                            # L2..L4
                            for li, nm in ((2, "w2"), (3, "w3"), (4, "w4")):
                                nhs = []
                                for m in range(2):
                                    ph = ps_h.tile([128, 512], f32, tag="ph")
                                    nc.tensor.matmul(ph[:, :], wk[(nm, 0)][:, m * 128:(m + 1) * 128],
                                                     hs[0][:, :], start=True, stop=False)
                                    nc.tensor.matmul(ph[:, :], wk[(nm, 1)][:, m * 128:(m + 1) * 128],
                                                     hs[1][:, :], start=False, stop=True)
                                    h = hpool.tile([128, 512], f16, tag="h")
                                    bap = bt[(f"b{li}", m)][:, :]
                                    if m == 0:
                                        nc.scalar.activation(h[:, :], ph[:, :], AF.Relu, bias=bap)
                                    else:
                                        nc.vector.tensor_scalar(h[:, :], ph[:, :], bap, 0.0,
                                                                OP.add, OP.max)
                                    nhs.append(h)
                                hs = nhs
                            # L5 -> pred f16
                            pr_all = ppool.tile([128, 4, 512], f16, tag="pr")
                            for m in range(4):
                                pp = ps_p.tile([128, 512], f32, tag="pp")
                                nc.tensor.matmul(pp[:, :], wk[("w5", 0)][:, m * 128:(m + 1) * 128],
                                                 hs[0][:, :], start=True, stop=False)
                                nc.tensor.matmul(pp[:, :], wk[("w5", 1)][:, m * 128:(m + 1) * 128],
                                                 hs[1][:, :], start=False, stop=True)
                                nc.scalar.activation(pr_all[:, m, :], pp[:, :], AF.Identity,
                                                     bias=bt[("b5", m)][:, :])

                            pend.append((qf_subs[t], t, pr_all, osb_m, mi))
                            if len(pend) > 1:
                                emit_dot(pend.pop(0))
                        # flush within the loop body (cross-iteration
                        # pipelining is blocked by the back-edge barrier)
                        while pend:
                            emit_dot(pend.pop(0))
    nc.finalize()
    return nc


def _prep_queries(coord, cell):
    """Voxel/table index + MLP input per query, f32 (reference arithmetic)."""
    cmu = coord - cell * 0.5
    eps = np.float32(1e-6)
    t1 = np.minimum(np.maximum(cmu + eps, np.float32(-1.0 + 1e-6)),
                    np.float32(1.0 - 1e-6))
    u = t1 * np.float32(16.0) + np.float32(15.5)
    ivox = np.round(u).astype(np.int32)          # RNE, matches HW convert
    iz, iy, ix = ivox[:, 0], ivox[:, 1], ivox[:, 2]
    lin = (iz * 1024 + (iy & 3) * 256 + ix * 8 + (iy >> 2)).astype(np.int16)

    # analytic q_coord (feature-center volume incl. the W-axis index shifts)
    upv = cmu * np.float32(16.0) + np.float32(15.5)
    ri = np.round(upv).astype(np.int32)
    rf = ri.astype(np.float32)
    val = np.all(rf >= 0, axis=1).astype(np.float32)
    rf = np.maximum(rf, np.float32(0.0))
    shv = ((rf[:, 2] < 2.0).astype(np.float32)
           + (rf[:, 2] == 3.0).astype(np.float32)) * np.float32(1.0 / 32.0)
    qcv = rf * np.float32(1.0 / 16.0) - np.float32(31.0 / 32.0)
    qcv = (qcv - shv[:, None]) * val[:, None]
    rel = (cmu - qcv) * np.float32(32.0)
    xin = np.concatenate([rel, cell[:, 0:1] * np.float32(16.0)], axis=1)
    return lin, xin.astype(np.float16)


def _prep_in_maps(inputs, qpad=QPAD, ncores=NCORES, qpc=None):
    coord = np.asarray(inputs["coord"], np.float32)[0]
    cell = np.asarray(inputs["cell"], np.float32)[0]
    blob16, b5p = _build_host_consts(
        inputs["inp"], inputs["W_enc"], inputs["b_enc"],
        inputs["W1"], inputs["W2"], inputs["W3"], inputs["W4"],
        inputs["W5"], inputs["b5"])
    bia = np.concatenate([
        np.asarray(inputs["b1"], np.float32).ravel(),
        np.asarray(inputs["b2"], np.float32).ravel(),
        np.asarray(inputs["b3"], np.float32).ravel(),
        np.asarray(inputs["b4"], np.float32).ravel(),
        b5p.ravel(),
    ]).astype(np.float32)
    assert bia.size == NBIA
    lin, xin = _prep_queries(coord, cell)
    if qpc is None:
        qpc = QTOT // ncores
    pad = qpad - qpc
    in_maps = []
    for c in range(ncores):
        lc = lin[c * qpc:(c + 1) * qpc]
        xc = xin[c * qpc:(c + 1) * qpc]
        lc = np.concatenate([lc, np.repeat(lc[-1:], pad, 0)], 0)
        xc = np.concatenate([xc, np.repeat(xc[-1:], pad, 0)], 0)
        # wrapped gather-index layout: idx[g*16+r, c*8+t] = lin[c*128+t*16+r]
        linm = lc.reshape(NMACRO, COLS, 8, 16)
        idxw = np.transpose(linm, (0, 3, 1, 2)).reshape(NMACRO, 16, COLS * 8)
        xinT = np.ascontiguousarray(
            xc.reshape(NMACRO, MACRO, 4).transpose(0, 2, 1))  # [nmacro,4,macro]
        in_maps.append({"b16p": np.ascontiguousarray(blob16[c * N16P:(c + 1) * N16P]),
                        "idx": np.ascontiguousarray(idxw),
                        "xin": xinT, "bia": bia})
    return in_maps


def kernel(**inputs):
    from concourse import bass_utils

    key = "full"
    if key not in _COMPILED:
        _COMPILED[key] = build_nc()
    nc = _COMPILED[key]
    in_maps = _prep_in_maps(inputs)
    res = bass_utils.run_bass_kernel_spmd(nc, in_maps, core_ids=list(range(NCORES)))
    outs = res.results
    qpc = QTOT // NCORES
    parts = [outs[c]["out"].reshape(-1)[:qpc] for c in range(NCORES)]
    return np.concatenate(parts).reshape(1, QTOT, 1).astype(np.float32)
